# revision 16
# baseline (speedup 1.0000x reference)
"""BitLinear (BitNet b1.58) kernel for 8x Trainium2 NeuronCores.

y = (round(x * 127/absmax(x)) @ unpack_ternary(weight).T) * weight_scale / (127/absmax(x))

Strategy (column-parallel / tensor-parallel over output features N):
  - Shard packed weight rows (N) across 8 cores; replicate activations.
  - On device: unpack the 2-bit ternary weights once into resident SBUF bf16
    (two's-complement decode: exact), quantize activations per-token to
    int8-valued bf16 (exact), and run the whole GEMM in bf16 with fp32 PSUM
    accumulation (exact: all products/sums are small integers < 2^24).
  - Host concatenates per-core outputs along N.
"""

import sys
import types
import functools

import numpy as np

# ---------------------------------------------------------------------------
# Problem constants (hardcoded; kernel.py must be self-contained)
# ---------------------------------------------------------------------------
B, S, K, N = 2, 2048, 4096, 16384
NCORES = 8
M = B * S                  # 4096 tokens
NSH = N // NCORES          # 2048 output features per core
P = 128
MAGIC = 12582912.0         # 1.5 * 2**23: float32 round-to-nearest-even bias


def _ensure_axon_hooks():
    """The container's antenv lacks axon_hooks; synthesize it so
    run_bass_kernel_spmd(trace=True) can register the NTFF profile hook."""
    if "antenv.axon_hooks" in sys.modules:
        return
    try:
        import antenv
    except ImportError:
        return
    m = types.ModuleType("antenv.axon_hooks")
    holder = [None]
    m.set_axon_ntff_profile_hook = lambda h: holder.__setitem__(0, h)
    m.get_axon_ntff_profile_hook = lambda: holder[0]
    sys.modules["antenv.axon_hooks"] = m
    antenv.axon_hooks = m
    try:
        from trn_agent_boot.trn_boot import _ntff_profile_via_ctypes

        m.set_axon_ntff_profile_hook(
            _ntff_profile_via_ctypes("/opt/axon/libaxon_pjrt.so")
        )
    except Exception:
        pass


@functools.lru_cache(maxsize=4)
def build_program(wsv: float, m_tokens: int = M, nsh: int = NSH, k: int = K):
    """Build the single-core SPMD Bass program.

    wsv: weight_scale[0] (baked as an immediate into the output scale).
    """
    import concourse.bass as bass  # noqa: F401
    import concourse.mybir as mybir
    import concourse.tile as tile
    from concourse import bacc
    from concourse.bass import ds
    from concourse.masks import make_identity

    f32 = mybir.dt.float32
    bf16 = mybir.dt.bfloat16
    fp8 = mybir.dt.float8e4
    i16 = mybir.dt.int16
    AF = mybir.ActivationFunctionType
    OP = mybir.AluOpType
    AX = mybir.AxisListType

    T = k // 1024            # k8-outer tiles of 128 partitions (4)
    J = 8                    # 2-bit lanes per uint16
    K8 = k // 8              # 512
    MT = m_tokens // P       # 32 m-tiles
    NT = nsh // 512          # 4 n-tiles

    nc = bacc.Bacc("TRN2", target_bir_lowering=False, debug=False,
                   num_devices=NCORES)
    x_d = nc.dram_tensor("x", [m_tokens, k], f32, kind="ExternalInput").ap()
    wp_d = nc.dram_tensor("wp", [nsh, K8], i16, kind="ExternalInput").ap()
    out_d = nc.dram_tensor("out", [m_tokens, nsh], f32,
                           kind="ExternalOutput").ap()

    with tile.TileContext(nc) as tc:
        from contextlib import ExitStack

        with ExitStack() as ctx:
            cpool = ctx.enter_context(tc.tile_pool(name="const", bufs=1))
            wpool = ctx.enter_context(tc.tile_pool(name="w", bufs=1))
            u16pool = ctx.enter_context(tc.tile_pool(name="u16", bufs=2))
            tmppool = ctx.enter_context(tc.tile_pool(name="tmp", bufs=3))
            xpool = ctx.enter_context(tc.tile_pool(name="x", bufs=2))
            xqpool = ctx.enter_context(tc.tile_pool(name="xq", bufs=1))
            xtpool = ctx.enter_context(tc.tile_pool(name="xt", bufs=2))
            opool = ctx.enter_context(tc.tile_pool(name="o", bufs=3))
            spool = ctx.enter_context(tc.tile_pool(name="s", bufs=2))
            pst = ctx.enter_context(
                tc.tile_pool(name="pst", bufs=3, space="PSUM"))
            psm = ctx.enter_context(
                tc.tile_pool(name="psm", bufs=2, space="PSUM"))

            ident = cpool.tile([P, P], bf16, name="ident")
            make_identity(nc, ident[:])

            # ---------------- weight prep (one-time) ----------------
            # packed u16 [nsh, K8] --transpose--> [K8, nsh] as T tiles of
            # [128, nsh]; partition p of tile t is k8 = 128*t + p.
            # lane j of u16 holds ternary code for k = 8*k8 + j.
            # fp8e4 holds {-1,0,1} exactly; fp8 rhs runs at bf16 PE speed
            # (no DoubleRow) and halves resident-weight SBUF.
            w_sb = [wpool.tile([P, J, nsh], fp8, name=f"wsb{t}")
                    for t in range(T)]
            u16t = []
            for t in range(T):
                u = u16pool.tile([P, nsh], i16, name="u16t", tag="u16t")
                nc.sync.dma_start(u[:], wp_d[:, ds(P * t, P)], transpose=True)
                u16t.append(u)
            # Emit mi=0 activation load/quant before the unpack so the PE
            # can start transposing early while unpack proceeds.

            def load_quant(mi):
                xs = xpool.tile([P, k], f32, name="xs", tag="xs")
                nc.sync.dma_start(xs[:], x_d[ds(P * mi, P), :])
                amax = spool.tile([P, 1], f32, name="amax", tag="amax")
                nc.vector.tensor_reduce(amax[:], xs[:], axis=AX.X, op=OP.max,
                                        apply_absolute_value=True)
                nc.vector.tensor_scalar_max(amax[:], amax[:], 1e-5)
                # s127 = 127 * (1/amax). No engine has an IEEE divide, so
                # this can differ from the reference's 127/amax by ~1 ulp,
                # flipping round(x*s) only when x*s sits within ~1 ulp of a
                # .5 boundary (couple per million values; |out| shift is one
                # quant step).
                rcp = spool.tile([P, 1], f32, name="rcp", tag="rcp")
                nc.vector.reciprocal(rcp[:], amax[:])
                s127 = spool.tile([P, 1], f32, name="s127", tag="s127")
                nc.vector.tensor_scalar_mul(s127[:], rcp[:], 127.0)
                oscale = spool.tile([P, 1], f32, name="oscale", tag="oscale")
                nc.vector.tensor_scalar_mul(oscale[:], amax[:], wsv / 127.0)
                # xq = RNE(x * s127) via the magic-number trick, all on DVE
                # (HW ACT rounds .5 ties differently from numpy; DVE is
                # exact fp32 RNE).
                xr = xpool.tile([P, k], f32, name="xr", tag="xr")
                nc.vector.tensor_scalar(xr[:], xs[:], s127[:], MAGIC,
                                        OP.mult, OP.add)
                xq = xqpool.tile([P, k], bf16, name="xq", tag="xq")
                nc.vector.tensor_scalar_add(xq[:], xr[:], -MAGIC)
                return xq, oscale

            xq0 = load_quant(0)

            # unpack lane j: t = (u << (14-2j)) & 0xC000 puts the 2-bit
            # field at [15:14]; as int16 that is 16384*decode(field) with
            # decode in {-2,-1,0,1} (two's complement). The ACT scaled copy
            # (x * 2^-14 -> fp8) finishes the exact decode.
            for t in range(T):
                for j in range(J):
                    tmp = tmppool.tile([P, nsh], i16, name="tmp", tag="tmp")
                    nc.vector.tensor_scalar(tmp[:], u16t[t][:],
                                            14 - 2 * j, -16384,
                                            OP.logical_shift_left,
                                            OP.bitwise_and)
                    nc.scalar.mul(w_sb[t][:, j, :], tmp[:], 2.0 ** -14)

            # ---------------- main loop ----------------
            qcache = xq0
            for mi in range(MT):
                xq, oscale = qcache
                # transpose xq [m, k] -> xqT [k8p, j, t, m] via PE
                xq_r = xq.rearrange("m (t p j) -> m t p j", t=T, j=J)
                xqT = xtpool.tile([P, J, T, P], bf16, name="xqT", tag="xqT")
                for t in range(T):
                    for j in range(J):
                        ps = pst.tile([P, P], f32, name="pst", tag="pst")
                        nc.tensor.matmul(ps[:], lhsT=xq_r[:, t, :, j],
                                         rhs=ident[:], start=True, stop=True)
                        nc.scalar.copy(xqT[:, j, t, :], ps[:])
                if mi + 1 < MT:
                    qcache = load_quant(mi + 1)
                for ni in range(NT):
                    ps = psm.tile([P, 512], f32, name="psm", tag="psm")
                    step = 0
                    for t in range(T):
                        for j in range(J):
                            nc.tensor.matmul(
                                ps[:], lhsT=xqT[:, j, t, :],
                                rhs=w_sb[t][:, j, ds(512 * ni, 512)],
                                start=(step == 0), stop=(step == T * J - 1))
                            step += 1
                    ot = opool.tile([P, 512], f32, name="ot", tag="ot")
                    nc.vector.tensor_scalar_mul(ot[:], ps[:], oscale[:])
                    nc.sync.dma_start(
                        out_d[ds(P * mi, P), ds(512 * ni, 512)], ot[:])

    nc.compile()
    return nc


def kernel(x: np.ndarray, weight: np.ndarray,
           weight_scale: np.ndarray) -> np.ndarray:
    """x: [B,S,K] f32; weight: [N, K//4] uint8 packed ternary;
    weight_scale: [4] f32 (replicated scalar). Returns [B,S,N] f32."""
    _ensure_axon_hooks()
    from concourse import bass_utils

    x2 = np.ascontiguousarray(np.asarray(x).reshape(M, K), dtype=np.float32)
    wp = np.ascontiguousarray(np.asarray(weight, dtype=np.uint8))
    wp16 = wp.view(np.int16)              # [N, K//8] little-endian pairs
    wsv = float(np.asarray(weight_scale).reshape(-1)[0])

    nc = build_program(wsv)
    in_maps = [
        {"x": x2, "wp": np.ascontiguousarray(wp16[c * NSH:(c + 1) * NSH])}
        for c in range(NCORES)
    ]
    res = bass_utils.run_bass_kernel_spmd(
        nc, in_maps, core_ids=list(range(NCORES)))
    out = np.concatenate(
        [res.results[c]["out"] for c in range(NCORES)], axis=1)
    return out.reshape(B, S, N)
